# revision 1
# baseline (speedup 1.0000x reference)
"""Trainium2 Bass kernel for FeatureOnlyGate MoE routing.

Math: g = h @ W.T + b  (h: [N,64], W: [6,64], b: [6])
      out = renormalized top-2 softmax of g per row:
        out[x] = sigmoid(g1-g2) at argmax, sigmoid(g2-g1) at arg-2nd, 0 else
      (the full-softmax denominator cancels after top-2 masking).

All math is full fp32: the measured min gap between 2nd and 3rd logit on
this problem's data is 2.5e-7, so any reduced-precision matmul (bf16,
fp32r) flips top-2 selections and produces O(0.3) pointwise errors.

Per-core dataflow (data parallel over 8 cores, 262144 tokens each):
  DMA h tiles [128 part, 512] (8 consecutive token rows = 2KB contiguous
  per partition) -> 4x PE pair-transpose [128,128] (two tokens' 64
  features stacked along K) -> ACT copy PSUM->SBUF -> per pair one fp32
  matmul with hT as the stationary operand and a block-diagonal glued
  gate weight [128,12] streaming -> logits land token-major [128, 12]
  directly in PSUM, 8 chunks packed per bank -> one DVE bias pass ->
  batched DVE top-2 mask + ACT tanh/sigmoid -> contiguous DMA out.
"""

import os
import numpy as np

N_FULL = 2097152
D = 64
E = 6
NCORES = 8
NSH = N_FULL // NCORES  # 262144 tokens per core

P = 128        # partitions
JT = 8         # consecutive tokens per partition per chunk
CHUNK = P * JT  # 1024 tokens
GROUP = 8      # chunks per psum_g bank (8 * 48 f32 = 384 of 512)
BATCH_G = 2    # groups per DVE batch
BATCH = GROUP * BATCH_G            # 16 chunks per batch
TOKB = CHUNK * BATCH               # 16384 tokens per batch
FDB = BATCH * JT * E               # 768 logit elems per partition per batch
TB = BATCH * JT                    # 128 tokens per partition per batch

LAST_RESULTS = None  # BassKernelResults of the last hardware run (for test.py)


def _build_nc(nsh, repeats=1, warm=False, warm_n=512):
    import concourse.bass as bass
    from concourse import bacc
    import concourse.mybir as mybir
    from concourse.tile import TileContext

    f32 = mybir.dt.float32
    Alu = mybir.AluOpType
    Act = mybir.ActivationFunctionType

    nbatch = nsh // TOKB
    assert nsh % TOKB == 0

    nc = bacc.Bacc(None)
    h = nc.declare_dram_parameter("h", [nsh, D], f32, isOutput=False)
    ident = nc.declare_dram_parameter("ident", [P, P], f32, isOutput=False)
    wglue = nc.declare_dram_parameter("wglue", [P, 2 * E], f32, isOutput=False)
    biasr = nc.declare_dram_parameter("biasr", [P, E], f32, isOutput=False)
    out = nc.declare_dram_parameter("out", [nsh, E], f32, isOutput=True)

    # token n = chunk*1024 + 8*p + j  ->  per chunk: [128 part, 512] with
    # 8 full token rows (2KB) contiguous per partition.
    hv = h[:, :].rearrange("(nc p j) e -> nc p (j e)", p=P, j=JT)
    # out per batch: [128 part, 16 chunk, 48] ; (j x) contiguous in DRAM.
    ov = out[:, :].rearrange(
        "(nb c p j) x -> nb p c (j x)", c=BATCH, p=P, j=JT
    )

    with TileContext(nc) as tc:
        with (
            tc.tile_pool(name="const_pool", bufs=1) as const_pool,
            tc.tile_pool(name="h_pool", bufs=4) as h_pool,
            tc.tile_pool(name="hT_psum", bufs=3, space="PSUM") as hT_psum_pool,
            tc.tile_pool(name="hT_sb", bufs=4) as hT_sb_pool,
            tc.tile_pool(name="g_psum", bufs=3, space="PSUM") as g_psum_pool,
            tc.tile_pool(name="t0_pool", bufs=2) as t0_pool,
            tc.tile_pool(name="work", bufs=2) as work,
        ):
            ident_sb = const_pool.tile([P, P], f32)
            wg_sb = const_pool.tile([P, 2 * E], f32)
            bias_sb = const_pool.tile([P, E], f32)
            nc.sync.dma_start(out=ident_sb[:, :], in_=ident[:, :])
            nc.sync.dma_start(out=wg_sb[:, :], in_=wglue[:, :])
            nc.sync.dma_start(out=bias_sb[:, :], in_=biasr[:, :])

            if warm:
                bf16 = mybir.dt.bfloat16
                dummy_w = const_pool.tile([P, P], bf16)
                dummy_x = const_pool.tile([P, 512], bf16)
                nc.vector.memset(dummy_w[:, :], 0.0)
                nc.vector.memset(dummy_x[:, :], 0.0)

                warm_ps = g_psum_pool.tile([P, 512], f32, tag="warm", bufs=1)
                for _ in range(40):
                    nc.tensor.matmul(
                        warm_ps[:, :], dummy_w[:, :], dummy_x[:, :],
                        start=True, stop=True,
                    )

            import contextlib

            loop_ctx = (
                tc.For_i(0, repeats, 1)
                if repeats > 1
                else contextlib.nullcontext()
            )
            with loop_ctx:
              for bi in range(nbatch):
                sb_t0 = t0_pool.tile([P, FDB], f32)
                for gi in range(BATCH_G):
                    psum_g = g_psum_pool.tile([P, 512], f32)
                    for k in range(GROUP):
                        c = (bi * BATCH_G + gi) * GROUP + k
                        h_sb = h_pool.tile([P, 512], f32)
                        nc.sync.dma_start(out=h_sb[:, :], in_=hv[c])
                        psum_hT = hT_psum_pool.tile([P, 512], f32)
                        for m in range(4):
                            sl = slice(128 * m, 128 * (m + 1))
                            nc.tensor.transpose(
                                psum_hT[:, sl],
                                h_sb[:, sl],
                                ident_sb[:, :],
                            )
                        sb_hT = hT_sb_pool.tile([P, 512], f32)
                        nc.scalar.activation(
                            sb_hT[:, :], psum_hT[:, :], Act.Copy
                        )
                        for m in range(4):
                            off = 48 * k + 12 * m
                            nc.tensor.matmul(
                                psum_g[:, off : off + 12],
                                sb_hT[:, 128 * m : 128 * (m + 1)],
                                wg_sb[:, :],
                                start=True,
                                stop=True,
                            )
                        if warm:
                            nc.tensor.matmul(
                                warm_ps[:, 0:warm_n],
                                dummy_w[:, :],
                                dummy_x[:, 0:warm_n],
                                start=True,
                                stop=True,
                            )
                    # bias add: psum group (8 chunks) -> compact sbuf
                    in3 = psum_g[:, 0 : GROUP * 48].rearrange(
                        "c (ch j x) -> c ch j x", ch=GROUP, j=JT
                    )
                    out3 = sb_t0[
                        :, gi * GROUP * 48 : (gi + 1) * GROUP * 48
                    ].rearrange("c (ch j x) -> c ch j x", ch=GROUP, j=JT)
                    bias3 = (
                        bias_sb[:, :]
                        .unsqueeze(1)
                        .unsqueeze(1)
                        .broadcast_to([P, GROUP, JT, E])
                    )
                    nc.vector.tensor_tensor(
                        out=out3, in0=in3, in1=bias3, op=Alu.add
                    )

                # ---- batched top-2 softmax over 16 chunks ----
                t3 = sb_t0[:, :].rearrange("c (t x) -> c t x", x=E)
                m1 = work.tile([P, TB], f32)
                nc.vector.tensor_reduce(
                    out=m1[:, :], in_=t3, axis=mybir.AxisListType.X, op=Alu.max
                )
                m1b = m1[:, :].unsqueeze(2).broadcast_to([P, TB, E])
                c1 = work.tile([P, FDB], f32)
                c13 = c1[:, :].rearrange("c (t x) -> c t x", x=E)
                nc.vector.tensor_tensor(out=c13, in0=t3, in1=m1b, op=Alu.is_ge)
                masked = work.tile([P, FDB], f32)
                mk3 = masked[:, :].rearrange("c (t x) -> c t x", x=E)
                nc.vector.scalar_tensor_tensor(
                    out=mk3, in0=c13, scalar=-1e30, in1=t3,
                    op0=Alu.mult, op1=Alu.add,
                )
                m2 = work.tile([P, TB], f32)
                nc.vector.tensor_reduce(
                    out=m2[:, :], in_=mk3, axis=mybir.AxisListType.X, op=Alu.max
                )
                m2b = m2[:, :].unsqueeze(2).broadcast_to([P, TB, E])
                mask2 = work.tile([P, FDB], f32)
                mask23 = mask2[:, :].rearrange("c (t x) -> c t x", x=E)
                nc.vector.tensor_tensor(out=mask23, in0=t3, in1=m2b, op=Alu.is_ge)
                dd = work.tile([P, TB], f32)
                nc.vector.tensor_tensor(
                    out=dd[:, :], in0=m1[:, :], in1=m2[:, :], op=Alu.subtract
                )
                qq = work.tile([P, TB], f32)
                nc.scalar.activation(qq[:, :], dd[:, :], Act.Tanh, scale=0.5)
                p2 = work.tile([P, TB], f32)
                nc.scalar.activation(p2[:, :], dd[:, :], Act.Sigmoid, scale=-1.0)
                qqb = qq[:, :].unsqueeze(2).broadcast_to([P, TB, E])
                p2b = p2[:, :].unsqueeze(2).broadcast_to([P, TB, E])
                o1 = work.tile([P, FDB], f32)
                o13 = o1[:, :].rearrange("c (t x) -> c t x", x=E)
                nc.vector.tensor_tensor(out=o13, in0=c13, in1=qqb, op=Alu.mult)
                o2 = work.tile([P, FDB], f32)
                o23 = o2[:, :].rearrange("c (t x) -> c t x", x=E)
                nc.vector.tensor_tensor(out=o23, in0=mask23, in1=p2b, op=Alu.mult)
                res = work.tile([P, FDB], f32)
                nc.vector.tensor_tensor(
                    out=res[:, :], in0=o1[:, :], in1=o2[:, :], op=Alu.add
                )
                nc.sync.dma_start(
                    out=ov[bi],
                    in_=res[:, :].rearrange("c (k q) -> c k q", k=BATCH),
                )

            if warm:
                warm_sink = const_pool.tile([P, 1], f32)
                nc.scalar.activation(
                    warm_sink[:, :], warm_ps[:, 0:1], Act.Copy
                )
                warm_dram = nc.dram_tensor("warm_sink_d", [P, 1], f32)
                nc.sync.dma_start(out=warm_dram[:, :], in_=warm_sink[:, :])

    nc.finalize()
    return nc


def _aux_inputs(W, b):
    ident = np.eye(P, dtype=np.float32)
    # wglue[64*bb + e, 6*bb' + x] = W[x, e] iff bb == bb'
    wglue = np.zeros((P, 2 * E), dtype=np.float32)
    wglue[0:D, 0:E] = W.T.astype(np.float32)
    wglue[D : 2 * D, E : 2 * E] = W.T.astype(np.float32)
    biasr = np.tile(b.astype(np.float32)[None, :], (P, 1))
    return ident, wglue, biasr


_NC_CACHE = {}


def _get_nc(nsh, repeats=1, warm=False, warm_n=512):
    key = (nsh, repeats, warm, warm_n)
    if key not in _NC_CACHE:
        _NC_CACHE[key] = _build_nc(nsh, repeats, warm, warm_n)
    return _NC_CACHE[key]


def kernel(h, W, b):
    global LAST_RESULTS
    from concourse.bass_utils import run_bass_kernel_spmd

    h = np.ascontiguousarray(np.asarray(h, dtype=np.float32))
    W = np.asarray(W, dtype=np.float32)
    b = np.asarray(b, dtype=np.float32)
    n = h.shape[0]
    nsh = n // NCORES
    nc = _get_nc(nsh, warm=True, warm_n=256)
    ident, wglue, biasr = _aux_inputs(W, b)
    in_maps = []
    for i in range(NCORES):
        in_maps.append(
            {
                "h": h[i * nsh : (i + 1) * nsh],
                "ident": ident,
                "wglue": wglue,
                "biasr": biasr,
            }
        )
    trace = bool(int(os.environ.get("KERNEL_TRACE", "0")))
    res = run_bass_kernel_spmd(
        nc, in_maps, list(range(NCORES)), trace=trace
    )
    LAST_RESULTS = res
    outs = [res.results[i]["out"] for i in range(NCORES)]
    return np.concatenate(outs, axis=0)



# revision 14
# speedup vs baseline: 19.8732x; 19.8732x over previous
"""Trainium2 Bass kernel for FeatureOnlyGate MoE routing.

Math: g = h @ W.T + b  (h: [N,64], W: [6,64], b: [6])
      out = renormalized top-2 softmax of g per row:
        out[x] = sigmoid(g1-g2) at argmax, sigmoid(g2-g1) at arg-2nd, 0 else
      (the full-softmax denominator cancels after top-2 masking).

All math is full fp32: the measured min gap between 2nd and 3rd logit on
this problem's data is 2.5e-7, so any reduced-precision matmul (bf16,
fp32r) flips top-2 selections and produces O(0.3) pointwise errors.

Per-core dataflow (data parallel over 8 cores, 262144 tokens each):
  Token map t = chunk*4096 + p*32 + j (p partition, j in-partition).
  DMA h chunks [128, 2048] (32 token rows = 8KB contiguous per
  partition) on the sync HWDGE queue -> per half-chunk 8x PE
  pair-transpose [128,128] into a 2-bank PSUM tile -> one wide ACT copy
  [128,1024] PSUM->SBUF -> per pair one fp32 matmul with hT stationary
  and a block-diagonal glued gate weight [128,12] streaming -> logits
  land token-major in PSUM [128,384] per 2 chunks -> DVE bias pass ->
  batched DVE top-2 mask + ACT tanh/sigmoid over 4-chunk batches ->
  output DMA (768B runs per partition) issued from the idle gpsimd
  queue so it never head-of-line blocks the input stream.
"""

import os
import numpy as np

N_FULL = 2097152
D = 64
E = 6
NCORES = 8
NSH = N_FULL // NCORES  # 262144 tokens per core

P = 128         # partitions
JT = 32         # consecutive tokens per partition per chunk
CHUNK = P * JT  # 4096 tokens
GROUP = 2       # chunks per psum_g tile (2 * 32 * 6 = 384 of 512)
BATCH = 4       # chunks per DVE batch
TOKB = CHUNK * BATCH               # 16384 tokens per batch
FDB = BATCH * JT * E               # 768 logit elems per partition per batch
TB = BATCH * JT                    # 128 tokens per partition per batch

LAST_RESULTS = None  # BassKernelResults of the last hardware run (for test.py)


def _build_nc(nsh, repeats=1, warm=False, warm_n=256):
    import concourse.bass as bass
    from concourse import bacc
    import concourse.mybir as mybir
    from concourse.tile import TileContext

    f32 = mybir.dt.float32
    Alu = mybir.AluOpType
    Act = mybir.ActivationFunctionType

    nbatch = nsh // TOKB
    assert nsh % TOKB == 0

    nc = bacc.Bacc(None)
    h = nc.declare_dram_parameter("h", [nsh, D], f32, isOutput=False)
    ident = nc.declare_dram_parameter("ident", [P, P], f32, isOutput=False)
    wglue = nc.declare_dram_parameter("wglue", [P, 2 * E], f32, isOutput=False)
    biasr = nc.declare_dram_parameter("biasr", [P, E], f32, isOutput=False)
    out = nc.declare_dram_parameter("out", [nsh, E], f32, isOutput=True)

    # token t = chunk*4096 + 32*p + j  ->  per chunk: [128 part, 2048] with
    # 32 full token rows (8KB) contiguous per partition.
    hv = h[:, :].rearrange("(nch p j) e -> nch p (j e)", p=P, j=JT)
    # out per batch: [128 part, 4 chunk, 192] ; (j x) = 768B contiguous.
    ov = out[:, :].rearrange(
        "(nb c p j) x -> nb p c (j x)", c=BATCH, p=P, j=JT
    )

    with TileContext(nc) as tc:
        with (
            tc.tile_pool(name="const_pool", bufs=1) as const_pool,
            tc.tile_pool(name="h_pool", bufs=6) as h_pool,
            tc.tile_pool(name="hT_psum", bufs=3, space="PSUM") as hT_psum_pool,
            tc.tile_pool(name="hT_sb", bufs=4) as hT_sb_pool,
            tc.tile_pool(name="g_psum", bufs=2, space="PSUM") as g_psum_pool,
            tc.tile_pool(name="t0_pool", bufs=3) as t0_pool,
            tc.tile_pool(name="work", bufs=2) as work,
            tc.tile_pool(name="res_pool", bufs=3) as res_pool,
        ):
            ident_sb = const_pool.tile([P, P], f32)
            wg_sb = const_pool.tile([P, 2 * E], f32)
            bias_sb = const_pool.tile([P, E], f32)
            nc.sync.dma_start(out=ident_sb[:, :], in_=ident[:, :])
            nc.sync.dma_start(out=wg_sb[:, :], in_=wglue[:, :])
            nc.sync.dma_start(out=bias_sb[:, :], in_=biasr[:, :])

            if warm:
                bf16 = mybir.dt.bfloat16
                dummy_w = const_pool.tile([P, P], bf16)
                dummy_x = const_pool.tile([P, 512], bf16)
                nc.vector.memset(dummy_w[:, :], 0.0)
                nc.vector.memset(dummy_x[:, :], 0.0)

                warm_ps = g_psum_pool.tile([P, P], f32, tag="warm", bufs=1)
                for _ in range(40):
                    nc.tensor.matmul(
                        warm_ps[:, :], dummy_w[:, :], dummy_x[:, 0:P],
                        start=True, stop=True,
                    )

            import contextlib

            def chain_prefix(st):
                # DVE-only: m1, c1, masked, m2, mask2, dd for chunk range
                # [lo, hi) of batch st["bi"]
                lo, hi = st["lo"], st["hi"]
                TB = (hi - lo) * JT
                FDB = TB * E
                t3 = st["t0"][:, lo * JT * E : hi * JT * E].rearrange(
                    "c (t x) -> c t x", x=E
                )
                m1 = work.tile([P, TB], f32)
                nc.vector.tensor_reduce(
                    out=m1[:, :], in_=t3, axis=mybir.AxisListType.X, op=Alu.max
                )
                m1b = m1[:, :].unsqueeze(2).broadcast_to([P, TB, E])
                c1 = work.tile([P, FDB], f32)
                c13 = c1[:, :].rearrange("c (t x) -> c t x", x=E)
                nc.vector.tensor_tensor(out=c13, in0=t3, in1=m1b, op=Alu.is_ge)
                masked = work.tile([P, FDB], f32)
                mk3 = masked[:, :].rearrange("c (t x) -> c t x", x=E)
                nc.vector.scalar_tensor_tensor(
                    out=mk3, in0=c13, scalar=-1e30, in1=t3,
                    op0=Alu.mult, op1=Alu.add,
                )
                m2 = work.tile([P, TB], f32)
                nc.vector.tensor_reduce(
                    out=m2[:, :], in_=mk3, axis=mybir.AxisListType.X, op=Alu.max
                )
                m2b = m2[:, :].unsqueeze(2).broadcast_to([P, TB, E])
                mask2 = work.tile([P, FDB], f32)
                mask23 = mask2[:, :].rearrange("c (t x) -> c t x", x=E)
                nc.vector.tensor_tensor(
                    out=mask23, in0=t3, in1=m2b, op=Alu.is_ge
                )
                dd = work.tile([P, TB], f32)
                nc.vector.tensor_tensor(
                    out=dd[:, :], in0=m1[:, :], in1=m2[:, :], op=Alu.subtract
                )
                st.update(c13=c13, mask23=mask23, dd=dd)

            def chain_suffix(st):
                # ACT tanh/sigmoid then DVE combine + gpsimd output DMA
                lo, hi = st["lo"], st["hi"]
                TB = (hi - lo) * JT
                FDB = TB * E
                dd = st["dd"]
                qq = work.tile([P, TB], f32)
                nc.scalar.activation(qq[:, :], dd[:, :], Act.Tanh, scale=0.5)
                p2 = work.tile([P, TB], f32)
                nc.scalar.activation(
                    p2[:, :], dd[:, :], Act.Sigmoid, scale=-1.0
                )
                qqb = qq[:, :].unsqueeze(2).broadcast_to([P, TB, E])
                p2b = p2[:, :].unsqueeze(2).broadcast_to([P, TB, E])
                o1 = work.tile([P, FDB], f32)
                o13 = o1[:, :].rearrange("c (t x) -> c t x", x=E)
                nc.vector.tensor_tensor(
                    out=o13, in0=st["c13"], in1=qqb, op=Alu.mult
                )
                res = res_pool.tile([P, FDB], f32)
                res3 = res[:, :].rearrange("c (t x) -> c t x", x=E)
                nc.vector.tensor_tensor(
                    out=res3, in0=st["mask23"], in1=p2b, op=Alu.mult
                )
                nc.vector.tensor_tensor(
                    out=res[:, :], in0=o1[:, :], in1=res[:, :], op=Alu.add
                )
                nc.gpsimd.dma_start(
                    out=ov[st["bi"]][:, lo:hi],
                    in_=res[:, :].rearrange("c (k q) -> c k q", k=hi - lo),
                )

            loop_ctx = (
                tc.For_i(0, repeats, 1)
                if repeats > 1
                else contextlib.nullcontext()
            )
            with loop_ctx:
              pending = None
              for bi in range(nbatch):
                sb_t0 = t0_pool.tile([P, FDB], f32)
                if pending is not None:
                    chain_prefix(pending)
                for gi in range(BATCH // GROUP):
                    psum_g = g_psum_pool.tile([P, GROUP * JT * E], f32)
                    for ci in range(GROUP):
                        c = (bi * (BATCH // GROUP) + gi) * GROUP + ci
                        h_sb = h_pool.tile([P, JT * D], f32)
                        nc.sync.dma_start(out=h_sb[:, :], in_=hv[c])
                        for q in range(2):
                            psum_hT = hT_psum_pool.tile([P, 1024], f32)
                            for m in range(8):
                                sl = slice(
                                    1024 * q + 128 * m,
                                    1024 * q + 128 * (m + 1),
                                )
                                nc.tensor.transpose(
                                    psum_hT[:, 128 * m : 128 * (m + 1)],
                                    h_sb[:, sl],
                                    ident_sb[:, :],
                                )
                            sb_hT = hT_sb_pool.tile([P, 1024], f32)
                            nc.scalar.activation(
                                sb_hT[:, :], psum_hT[:, :], Act.Copy
                            )
                            for m in range(8):
                                i = q * 8 + m
                                off = ci * JT * E + 12 * i
                                nc.tensor.matmul(
                                    psum_g[:, off : off + 12],
                                    sb_hT[:, 128 * m : 128 * (m + 1)],
                                    wg_sb[:, :],
                                    start=True,
                                    stop=True,
                                )
                            if warm:
                                nc.tensor.matmul(
                                    warm_ps[:, 0 : min(warm_n, P)],
                                    dummy_w[:, :],
                                    dummy_x[:, 0 : min(warm_n, P)],
                                    start=True,
                                    stop=True,
                                )
                    # bias add: psum group (2 chunks) -> compact sbuf
                    gt = GROUP * JT  # 64 tokens per partition per group
                    in3 = psum_g[:, :].rearrange("c (t x) -> c t x", x=E)
                    out3 = sb_t0[
                        :, gi * gt * E : (gi + 1) * gt * E
                    ].rearrange("c (t x) -> c t x", x=E)
                    bias3 = (
                        bias_sb[:, :]
                        .unsqueeze(1)
                        .broadcast_to([P, gt, E])
                    )
                    nc.vector.tensor_tensor(
                        out=out3, in0=in3, in1=bias3, op=Alu.add
                    )

                    if gi == 0 and pending is not None:
                        chain_suffix(pending)
                        pending = None

                pending = {"bi": bi, "t0": sb_t0, "lo": 0, "hi": BATCH}
              if pending is not None:
                # Final batch: two half-width chains so the first half's
                # top-2 runs while the second half's matmuls still stream.
                lo_st = dict(pending, lo=0, hi=BATCH // 2)
                hi_st = dict(pending, lo=BATCH // 2, hi=BATCH)
                chain_prefix(lo_st)
                chain_suffix(lo_st)
                chain_prefix(hi_st)
                chain_suffix(hi_st)

            if warm:
                warm_sink = const_pool.tile([P, 1], f32)
                nc.scalar.activation(
                    warm_sink[:, :], warm_ps[:, 0:1], Act.Copy
                )
                warm_dram = nc.dram_tensor("warm_sink_d", [P, 1], f32)
                nc.sync.dma_start(out=warm_dram[:, :], in_=warm_sink[:, :])

    nc.finalize()
    return nc


def _aux_inputs(W, b):
    ident = np.eye(P, dtype=np.float32)
    # wglue[64*bb + e, 6*bb' + x] = W[x, e] iff bb == bb'
    wglue = np.zeros((P, 2 * E), dtype=np.float32)
    wglue[0:D, 0:E] = W.T.astype(np.float32)
    wglue[D : 2 * D, E : 2 * E] = W.T.astype(np.float32)
    biasr = np.tile(b.astype(np.float32)[None, :], (P, 1))
    return ident, wglue, biasr


_NC_CACHE = {}


def _get_nc(nsh, repeats=1, warm=False, warm_n=256):
    key = (nsh, repeats, warm, warm_n)
    if key not in _NC_CACHE:
        _NC_CACHE[key] = _build_nc(nsh, repeats, warm, warm_n)
    return _NC_CACHE[key]


def kernel(h, W, b):
    global LAST_RESULTS
    from concourse.bass_utils import run_bass_kernel_spmd

    h = np.ascontiguousarray(np.asarray(h, dtype=np.float32))
    W = np.asarray(W, dtype=np.float32)
    b = np.asarray(b, dtype=np.float32)
    n = h.shape[0]
    nsh = n // NCORES
    nc = _get_nc(nsh, warm=True, warm_n=256)
    ident, wglue, biasr = _aux_inputs(W, b)
    in_maps = []
    for i in range(NCORES):
        in_maps.append(
            {
                "h": h[i * nsh : (i + 1) * nsh],
                "ident": ident,
                "wglue": wglue,
                "biasr": biasr,
            }
        )
    trace = bool(int(os.environ.get("KERNEL_TRACE", "0")))
    res = run_bass_kernel_spmd(
        nc, in_maps, list(range(NCORES)), trace=trace
    )
    LAST_RESULTS = res
    outs = [res.results[i]["out"] for i in range(NCORES)]
    return np.concatenate(outs, axis=0)


# revision 25
# speedup vs baseline: 20.2299x; 1.0180x over previous
"""Trainium2 Bass kernel for FeatureOnlyGate MoE routing.

Math: g = h @ W.T + b  (h: [N,64], W: [6,64], b: [6])
      out = renormalized top-2 softmax of g per row:
        out[x] = sigmoid(g1-g2) at argmax, sigmoid(g2-g1) at arg-2nd, 0 else
      (the full-softmax denominator cancels after top-2 masking).

All math is full fp32: the measured min gap between 2nd and 3rd logit on
this problem's data is 2.5e-7, so any reduced-precision matmul (bf16,
fp32r) flips top-2 selections and produces O(0.3) pointwise errors.

Per-core dataflow (data parallel over 8 cores, 262144 tokens each):
  Token map t = chunk*4096 + p*32 + j (p partition, j in-partition).
  DMA h chunks [128, 2048] (32 token rows = 8KB contiguous per
  partition) on the sync HWDGE queue -> per half-chunk 8x PE
  pair-transpose [128,128] into a 2-bank PSUM tile -> one wide ACT copy
  [128,1024] PSUM->SBUF -> per pair one fp32 matmul with hT stationary
  and a block-diagonal glued gate weight [128,12] streaming -> logits
  land token-major in PSUM [128,384] per 2 chunks -> DVE bias pass ->
  batched DVE top-2 mask + ACT tanh/sigmoid over 4-chunk batches ->
  output DMA (768B runs per partition) issued from the idle gpsimd
  queue so it never head-of-line blocks the input stream.
"""

import os
import numpy as np

N_FULL = 2097152
D = 64
E = 6
NCORES = 8
NSH = N_FULL // NCORES  # 262144 tokens per core

P = 128         # partitions
JT = 32         # consecutive tokens per partition per chunk
CHUNK = P * JT  # 4096 tokens
GROUP = 2       # chunks per psum_g tile (2 * 32 * 6 = 384 of 512)
BATCH = 4       # chunks per DVE batch
TOKB = CHUNK * BATCH               # 16384 tokens per batch
FDB = BATCH * JT * E               # 768 logit elems per partition per batch
TB = BATCH * JT                    # 128 tokens per partition per batch

LAST_RESULTS = None  # BassKernelResults of the last hardware run (for test.py)

DEFAULT_CFG = dict(
    h_bufs=4,        # h_pool depth (chunks of 1 MB)
    dma_halves=1,    # input dma_starts per chunk (1 or 2)
    final_split=2,   # final batch emitted as this many sub-chains
    out_queue="gpsimd",  # engine issuing output DMAs
    use_select=False,  # DVE select() combine instead of o1/o2/add
    pool_chains=False,  # alternate top-2 chains between DVE and GpSimd
)


def _build_nc(nsh, repeats=1, warm=False, warm_n=256, cfg=None):
    cfg = dict(DEFAULT_CFG, **(cfg or {}))
    import concourse.bass as bass
    from concourse import bacc
    import concourse.mybir as mybir
    from concourse.tile import TileContext

    f32 = mybir.dt.float32
    Alu = mybir.AluOpType
    Act = mybir.ActivationFunctionType

    nbatch = nsh // TOKB
    assert nsh % TOKB == 0

    nc = bacc.Bacc(None)
    h = nc.declare_dram_parameter("h", [nsh, D], f32, isOutput=False)
    ident = nc.declare_dram_parameter("ident", [P, P], f32, isOutput=False)
    wglue = nc.declare_dram_parameter("wglue", [P, 2 * E], f32, isOutput=False)
    biasr = nc.declare_dram_parameter("biasr", [P, E], f32, isOutput=False)
    out = nc.declare_dram_parameter("out", [nsh, E], f32, isOutput=True)

    # token t = chunk*4096 + 32*p + j  ->  per chunk: [128 part, 2048] with
    # 32 full token rows (8KB) contiguous per partition.
    hv = h[:, :].rearrange("(nch p j) e -> nch p (j e)", p=P, j=JT)
    # out per batch: [128 part, 4 chunk, 192] ; (j x) = 768B contiguous.
    ov = out[:, :].rearrange(
        "(nb c p j) x -> nb p c (j x)", c=BATCH, p=P, j=JT
    )

    with TileContext(nc) as tc:
        with (
            tc.tile_pool(name="const_pool", bufs=1) as const_pool,
            tc.tile_pool(name="h_pool", bufs=cfg["h_bufs"]) as h_pool,
            tc.tile_pool(name="hT_psum", bufs=2, space="PSUM") as hT_psum_pool,
            tc.tile_pool(name="hT_sb", bufs=6) as hT_sb_pool,
            tc.tile_pool(name="g_psum", bufs=3, space="PSUM") as g_psum_pool,
            tc.tile_pool(name="t0_pool", bufs=3) as t0_pool,
            tc.tile_pool(name="work", bufs=3) as work,
            tc.tile_pool(name="res_pool", bufs=3) as res_pool,
        ):
            ident_sb = const_pool.tile([P, P], f32)
            wg_sb = const_pool.tile([P, 2 * E], f32)
            bias_sb = const_pool.tile([P, E], f32)
            nc.scalar.dma_start(out=ident_sb[:, :], in_=ident[:, :])
            nc.scalar.dma_start(out=wg_sb[:, :], in_=wglue[:, :])
            nc.scalar.dma_start(out=bias_sb[:, :], in_=biasr[:, :])

            if warm:
                bf16 = mybir.dt.bfloat16
                dummy_w = const_pool.tile([P, P], bf16)
                dummy_x = const_pool.tile([P, 512], bf16)
                nc.vector.memset(dummy_w[:, :], 0.0)
                nc.vector.memset(dummy_x[:, :], 0.0)

                warm_ps = g_psum_pool.tile([P, P], f32, tag="warm", bufs=1)
                for _ in range(40):
                    nc.tensor.matmul(
                        warm_ps[:, :], dummy_w[:, :], dummy_x[:, 0:P],
                        start=True, stop=True,
                    )

            import contextlib

            def chain_prefix(st):
                # vector-engine m1, c1, masked, m2, mask2, dd for chunk
                # range [lo, hi) of batch st["bi"]
                ve = st.get("ve", nc.vector)
                lo, hi = st["lo"], st["hi"]
                TB = (hi - lo) * JT
                FDB = TB * E
                t3 = st["t0"][:, lo * JT * E : hi * JT * E].rearrange(
                    "c (t x) -> c t x", x=E
                )
                m1 = work.tile([P, TB], f32)
                nc.vector.tensor_reduce(
                    out=m1[:, :], in_=t3, axis=mybir.AxisListType.X, op=Alu.max
                )
                m1b = m1[:, :].unsqueeze(2).broadcast_to([P, TB, E])
                c1 = work.tile([P, FDB], f32)
                c13 = c1[:, :].rearrange("c (t x) -> c t x", x=E)
                ve.tensor_tensor(out=c13, in0=t3, in1=m1b, op=Alu.is_ge)
                masked = work.tile([P, FDB], f32)
                mk3 = masked[:, :].rearrange("c (t x) -> c t x", x=E)
                ve.scalar_tensor_tensor(
                    out=mk3, in0=c13, scalar=-1e30, in1=t3,
                    op0=Alu.mult, op1=Alu.add,
                )
                m2 = work.tile([P, TB], f32)
                nc.vector.tensor_reduce(
                    out=m2[:, :], in_=mk3, axis=mybir.AxisListType.X, op=Alu.max
                )
                m2b = m2[:, :].unsqueeze(2).broadcast_to([P, TB, E])
                mask2 = work.tile([P, FDB], f32)
                mask23 = mask2[:, :].rearrange("c (t x) -> c t x", x=E)
                ve.tensor_tensor(
                    out=mask23, in0=t3, in1=m2b, op=Alu.is_ge
                )
                dd = work.tile([P, TB], f32)
                ve.tensor_tensor(
                    out=dd[:, :], in0=m1[:, :], in1=m2[:, :], op=Alu.subtract
                )
                st.update(c13=c13, c1f=c1[:, :], mask23=mask23, dd=dd)

            def chain_suffix(st):
                # ACT tanh/sigmoid then vector-engine combine + output DMA
                ve = st.get("ve", nc.vector)
                lo, hi = st["lo"], st["hi"]
                TB = (hi - lo) * JT
                FDB = TB * E
                dd = st["dd"]
                res = res_pool.tile([P, FDB], f32)
                res3 = res[:, :].rearrange("c (t x) -> c t x", x=E)
                if cfg["use_select"]:
                    s1 = work.tile([P, TB], f32)
                    nc.scalar.activation(
                        s1[:, :], dd[:, :], Act.Sigmoid, scale=1.0
                    )
                    p2 = work.tile([P, TB], f32)
                    nc.scalar.activation(
                        p2[:, :], dd[:, :], Act.Sigmoid, scale=-1.0
                    )
                    s1b = s1[:, :].unsqueeze(2).broadcast_to([P, TB, E])
                    p2b = p2[:, :].unsqueeze(2).broadcast_to([P, TB, E])
                    nc.vector.tensor_tensor(
                        out=res3, in0=st["mask23"], in1=p2b, op=Alu.mult
                    )
                    nc.vector.copy_predicated(
                        out=res3, mask=st["c1f"], data=s1b
                    )
                else:
                    qq = work.tile([P, TB], f32)
                    nc.scalar.activation(
                        qq[:, :], dd[:, :], Act.Tanh, scale=0.5
                    )
                    p2 = work.tile([P, TB], f32)
                    nc.scalar.activation(
                        p2[:, :], dd[:, :], Act.Sigmoid, scale=-1.0
                    )
                    qqb = qq[:, :].unsqueeze(2).broadcast_to([P, TB, E])
                    p2b = p2[:, :].unsqueeze(2).broadcast_to([P, TB, E])
                    o1 = work.tile([P, FDB], f32)
                    o13 = o1[:, :].rearrange("c (t x) -> c t x", x=E)
                    ve.tensor_tensor(
                        out=o13, in0=st["c13"], in1=qqb, op=Alu.mult
                    )
                    ve.tensor_tensor(
                        out=res3, in0=st["mask23"], in1=p2b, op=Alu.mult
                    )
                    ve.tensor_tensor(
                        out=res[:, :], in0=o1[:, :], in1=res[:, :], op=Alu.add
                    )
                out_eng = getattr(nc, cfg["out_queue"])
                out_eng.dma_start(
                    out=ov[st["bi"]][:, lo:hi],
                    in_=res[:, :].rearrange("c (k q) -> c k q", k=hi - lo),
                )

            loop_ctx = (
                tc.For_i(0, repeats, 1)
                if repeats > 1
                else contextlib.nullcontext()
            )
            with loop_ctx:
              pending = None
              for bi in range(nbatch):
                sb_t0 = t0_pool.tile([P, FDB], f32)
                if pending is not None:
                    chain_prefix(pending)
                for gi in range(BATCH // GROUP):
                    psum_g = g_psum_pool.tile([P, GROUP * JT * E], f32)
                    for ci in range(GROUP):
                        c = (bi * (BATCH // GROUP) + gi) * GROUP + ci
                        h_sb = h_pool.tile([P, JT * D], f32)
                        if cfg["dma_halves"] == 2:
                            nc.sync.dma_start(
                                out=h_sb[:, 0:1024], in_=hv[c][:, 0:1024]
                            )
                            nc.sync.dma_start(
                                out=h_sb[:, 1024:2048], in_=hv[c][:, 1024:2048]
                            )
                        else:
                            nc.sync.dma_start(out=h_sb[:, :], in_=hv[c])
                        for q in range(2):
                            psum_hT = hT_psum_pool.tile([P, 1024], f32)
                            for m in range(8):
                                sl = slice(
                                    1024 * q + 128 * m,
                                    1024 * q + 128 * (m + 1),
                                )
                                nc.tensor.transpose(
                                    psum_hT[:, 128 * m : 128 * (m + 1)],
                                    h_sb[:, sl],
                                    ident_sb[:, :],
                                )
                            sb_hT = hT_sb_pool.tile([P, 1024], f32)
                            nc.scalar.activation(
                                sb_hT[:, :], psum_hT[:, :], Act.Copy
                            )
                            for m in range(8):
                                i = q * 8 + m
                                off = ci * JT * E + 12 * i
                                nc.tensor.matmul(
                                    psum_g[:, off : off + 12],
                                    sb_hT[:, 128 * m : 128 * (m + 1)],
                                    wg_sb[:, :],
                                    start=True,
                                    stop=True,
                                )
                            if warm:
                                nc.tensor.matmul(
                                    warm_ps[:, 0 : min(warm_n, P)],
                                    dummy_w[:, :],
                                    dummy_x[:, 0 : min(warm_n, P)],
                                    start=True,
                                    stop=True,
                                )
                    # bias add: psum group (2 chunks) -> compact sbuf
                    gt = GROUP * JT  # 64 tokens per partition per group
                    in3 = psum_g[:, :].rearrange("c (t x) -> c t x", x=E)
                    out3 = sb_t0[
                        :, gi * gt * E : (gi + 1) * gt * E
                    ].rearrange("c (t x) -> c t x", x=E)
                    bias3 = (
                        bias_sb[:, :]
                        .unsqueeze(1)
                        .broadcast_to([P, gt, E])
                    )
                    nc.vector.tensor_tensor(
                        out=out3, in0=in3, in1=bias3, op=Alu.add
                    )

                    if gi == 0 and pending is not None:
                        chain_suffix(pending)
                        pending = None

                ve = (
                    nc.gpsimd
                    if (cfg["pool_chains"] and bi % 2 == 1)
                    else nc.vector
                )
                pending = {
                    "bi": bi, "t0": sb_t0, "lo": 0, "hi": BATCH, "ve": ve,
                }
              if pending is not None:
                # Final batch: split into sub-chains so earlier chunks'
                # top-2 runs while later chunks' matmuls still stream.
                step = BATCH // cfg["final_split"]
                for ck in range(0, BATCH, step):
                    st = dict(pending, lo=ck, hi=ck + step)
                    chain_prefix(st)
                    chain_suffix(st)

            if warm:
                warm_sink = const_pool.tile([P, 1], f32)
                nc.scalar.activation(
                    warm_sink[:, :], warm_ps[:, 0:1], Act.Copy
                )
                warm_dram = nc.dram_tensor("warm_sink_d", [P, 1], f32)
                nc.sync.dma_start(out=warm_dram[:, :], in_=warm_sink[:, :])

    nc.finalize()
    return nc


def _aux_inputs(W, b):
    ident = np.eye(P, dtype=np.float32)
    # wglue[64*bb + e, 6*bb' + x] = W[x, e] iff bb == bb'
    wglue = np.zeros((P, 2 * E), dtype=np.float32)
    wglue[0:D, 0:E] = W.T.astype(np.float32)
    wglue[D : 2 * D, E : 2 * E] = W.T.astype(np.float32)
    biasr = np.tile(b.astype(np.float32)[None, :], (P, 1))
    return ident, wglue, biasr


_NC_CACHE = {}


def _get_nc(nsh, repeats=1, warm=False, warm_n=256, cfg=None):
    key = (nsh, repeats, warm, warm_n,
           tuple(sorted((cfg or {}).items())))
    if key not in _NC_CACHE:
        _NC_CACHE[key] = _build_nc(nsh, repeats, warm, warm_n, cfg)
    return _NC_CACHE[key]


def kernel(h, W, b):
    global LAST_RESULTS
    from concourse.bass_utils import run_bass_kernel_spmd

    h = np.ascontiguousarray(np.asarray(h, dtype=np.float32))
    W = np.asarray(W, dtype=np.float32)
    b = np.asarray(b, dtype=np.float32)
    n = h.shape[0]
    nsh = n // NCORES
    nc = _get_nc(nsh, warm=True, warm_n=256)
    ident, wglue, biasr = _aux_inputs(W, b)
    in_maps = []
    for i in range(NCORES):
        in_maps.append(
            {
                "h": h[i * nsh : (i + 1) * nsh],
                "ident": ident,
                "wglue": wglue,
                "biasr": biasr,
            }
        )
    trace = bool(int(os.environ.get("KERNEL_TRACE", "0")))
    res = run_bass_kernel_spmd(
        nc, in_maps, list(range(NCORES)), trace=trace
    )
    LAST_RESULTS = res
    outs = [res.results[i]["out"] for i in range(NCORES)]
    return np.concatenate(outs, axis=0)


# revision 26
# speedup vs baseline: 20.2903x; 1.0030x over previous
"""Trainium2 Bass kernel for FeatureOnlyGate MoE routing.

Math: g = h @ W.T + b  (h: [N,64], W: [6,64], b: [6])
      out = renormalized top-2 softmax of g per row:
        out[x] = sigmoid(g1-g2) at argmax, sigmoid(g2-g1) at arg-2nd, 0 else
      (the full-softmax denominator cancels after top-2 masking).

All math is full fp32: the measured min gap between 2nd and 3rd logit on
this problem's data is 2.5e-7, so any reduced-precision matmul (bf16,
fp32r) flips top-2 selections and produces O(0.3) pointwise errors.

Per-core dataflow (data parallel over 8 cores, 262144 tokens each):
  Token map t = chunk*4096 + p*32 + j (p partition, j in-partition).
  DMA h chunks [128, 2048] (32 token rows = 8KB contiguous per
  partition) on the sync HWDGE queue -> per half-chunk 8x PE
  pair-transpose [128,128] into a 2-bank PSUM tile -> one wide ACT copy
  [128,1024] PSUM->SBUF -> per pair one fp32 matmul with hT stationary
  and a block-diagonal glued gate weight [128,12] streaming -> logits
  land token-major in PSUM [128,384] per 2 chunks -> DVE bias pass ->
  batched DVE top-2 mask + ACT tanh/sigmoid over 4-chunk batches ->
  output DMA (768B runs per partition) issued from the idle gpsimd
  queue so it never head-of-line blocks the input stream.
"""

import os
import numpy as np

N_FULL = 2097152
D = 64
E = 6
NCORES = 8
NSH = N_FULL // NCORES  # 262144 tokens per core

P = 128         # partitions
JT = 32         # consecutive tokens per partition per chunk
CHUNK = P * JT  # 4096 tokens
GROUP = 2       # chunks per psum_g tile (2 * 32 * 6 = 384 of 512)
BATCH = 4       # chunks per DVE batch
TOKB = CHUNK * BATCH               # 16384 tokens per batch
FDB = BATCH * JT * E               # 768 logit elems per partition per batch
TB = BATCH * JT                    # 128 tokens per partition per batch

LAST_RESULTS = None  # BassKernelResults of the last hardware run (for test.py)

DEFAULT_CFG = dict(
    h_bufs=4,        # h_pool depth (chunks of 1 MB)
    dma_halves=1,    # input dma_starts per chunk (1 or 2)
    final_split=2,   # final batch emitted as this many sub-chains
    out_queue="gpsimd",  # engine issuing output DMAs
    use_select=False,  # DVE select() combine instead of o1/o2/add
    pool_chains=False,  # alternate top-2 chains between DVE and GpSimd
    final2_split=True,  # split the second-to-last batch's chain as well
)


def _build_nc(nsh, repeats=1, warm=False, warm_n=256, cfg=None):
    cfg = dict(DEFAULT_CFG, **(cfg or {}))
    import concourse.bass as bass
    from concourse import bacc
    import concourse.mybir as mybir
    from concourse.tile import TileContext

    f32 = mybir.dt.float32
    Alu = mybir.AluOpType
    Act = mybir.ActivationFunctionType

    nbatch = nsh // TOKB
    assert nsh % TOKB == 0

    nc = bacc.Bacc(None)
    h = nc.declare_dram_parameter("h", [nsh, D], f32, isOutput=False)
    ident = nc.declare_dram_parameter("ident", [P, P], f32, isOutput=False)
    wglue = nc.declare_dram_parameter("wglue", [P, 2 * E], f32, isOutput=False)
    biasr = nc.declare_dram_parameter("biasr", [P, E], f32, isOutput=False)
    out = nc.declare_dram_parameter("out", [nsh, E], f32, isOutput=True)

    # token t = chunk*4096 + 32*p + j  ->  per chunk: [128 part, 2048] with
    # 32 full token rows (8KB) contiguous per partition.
    hv = h[:, :].rearrange("(nch p j) e -> nch p (j e)", p=P, j=JT)
    # out per batch: [128 part, 4 chunk, 192] ; (j x) = 768B contiguous.
    ov = out[:, :].rearrange(
        "(nb c p j) x -> nb p c (j x)", c=BATCH, p=P, j=JT
    )

    with TileContext(nc) as tc:
        with (
            tc.tile_pool(name="const_pool", bufs=1) as const_pool,
            tc.tile_pool(name="h_pool", bufs=cfg["h_bufs"]) as h_pool,
            tc.tile_pool(name="hT_psum", bufs=2, space="PSUM") as hT_psum_pool,
            tc.tile_pool(name="hT_sb", bufs=6) as hT_sb_pool,
            tc.tile_pool(name="g_psum", bufs=3, space="PSUM") as g_psum_pool,
            tc.tile_pool(name="t0_pool", bufs=3) as t0_pool,
            tc.tile_pool(name="work", bufs=3) as work,
            tc.tile_pool(name="res_pool", bufs=3) as res_pool,
        ):
            ident_sb = const_pool.tile([P, P], f32)
            wg_sb = const_pool.tile([P, 2 * E], f32)
            bias_sb = const_pool.tile([P, E], f32)
            nc.scalar.dma_start(out=ident_sb[:, :], in_=ident[:, :])
            nc.scalar.dma_start(out=wg_sb[:, :], in_=wglue[:, :])
            nc.scalar.dma_start(out=bias_sb[:, :], in_=biasr[:, :])

            if warm:
                bf16 = mybir.dt.bfloat16
                dummy_w = const_pool.tile([P, P], bf16)
                dummy_x = const_pool.tile([P, 512], bf16)
                nc.vector.memset(dummy_w[:, :], 0.0)
                nc.vector.memset(dummy_x[:, :], 0.0)

                warm_ps = g_psum_pool.tile([P, P], f32, tag="warm", bufs=1)
                for _ in range(40):
                    nc.tensor.matmul(
                        warm_ps[:, :], dummy_w[:, :], dummy_x[:, 0:P],
                        start=True, stop=True,
                    )

            import contextlib

            def chain_prefix(st):
                # vector-engine m1, c1, masked, m2, mask2, dd for chunk
                # range [lo, hi) of batch st["bi"]
                ve = st.get("ve", nc.vector)
                lo, hi = st["lo"], st["hi"]
                TB = (hi - lo) * JT
                FDB = TB * E
                t3 = st["t0"][:, lo * JT * E : hi * JT * E].rearrange(
                    "c (t x) -> c t x", x=E
                )
                m1 = work.tile([P, TB], f32)
                nc.vector.tensor_reduce(
                    out=m1[:, :], in_=t3, axis=mybir.AxisListType.X, op=Alu.max
                )
                m1b = m1[:, :].unsqueeze(2).broadcast_to([P, TB, E])
                c1 = work.tile([P, FDB], f32)
                c13 = c1[:, :].rearrange("c (t x) -> c t x", x=E)
                ve.tensor_tensor(out=c13, in0=t3, in1=m1b, op=Alu.is_ge)
                masked = work.tile([P, FDB], f32)
                mk3 = masked[:, :].rearrange("c (t x) -> c t x", x=E)
                ve.scalar_tensor_tensor(
                    out=mk3, in0=c13, scalar=-1e30, in1=t3,
                    op0=Alu.mult, op1=Alu.add,
                )
                m2 = work.tile([P, TB], f32)
                nc.vector.tensor_reduce(
                    out=m2[:, :], in_=mk3, axis=mybir.AxisListType.X, op=Alu.max
                )
                m2b = m2[:, :].unsqueeze(2).broadcast_to([P, TB, E])
                mask2 = work.tile([P, FDB], f32)
                mask23 = mask2[:, :].rearrange("c (t x) -> c t x", x=E)
                ve.tensor_tensor(
                    out=mask23, in0=t3, in1=m2b, op=Alu.is_ge
                )
                dd = work.tile([P, TB], f32)
                ve.tensor_tensor(
                    out=dd[:, :], in0=m1[:, :], in1=m2[:, :], op=Alu.subtract
                )
                st.update(c13=c13, c1f=c1[:, :], mask23=mask23, dd=dd)

            def chain_suffix(st):
                # ACT tanh/sigmoid then vector-engine combine + output DMA
                ve = st.get("ve", nc.vector)
                lo, hi = st["lo"], st["hi"]
                TB = (hi - lo) * JT
                FDB = TB * E
                dd = st["dd"]
                res = res_pool.tile([P, FDB], f32)
                res3 = res[:, :].rearrange("c (t x) -> c t x", x=E)
                if cfg["use_select"]:
                    s1 = work.tile([P, TB], f32)
                    nc.scalar.activation(
                        s1[:, :], dd[:, :], Act.Sigmoid, scale=1.0
                    )
                    p2 = work.tile([P, TB], f32)
                    nc.scalar.activation(
                        p2[:, :], dd[:, :], Act.Sigmoid, scale=-1.0
                    )
                    s1b = s1[:, :].unsqueeze(2).broadcast_to([P, TB, E])
                    p2b = p2[:, :].unsqueeze(2).broadcast_to([P, TB, E])
                    nc.vector.tensor_tensor(
                        out=res3, in0=st["mask23"], in1=p2b, op=Alu.mult
                    )
                    nc.vector.copy_predicated(
                        out=res3, mask=st["c1f"], data=s1b
                    )
                else:
                    qq = work.tile([P, TB], f32)
                    nc.scalar.activation(
                        qq[:, :], dd[:, :], Act.Tanh, scale=0.5
                    )
                    p2 = work.tile([P, TB], f32)
                    nc.scalar.activation(
                        p2[:, :], dd[:, :], Act.Sigmoid, scale=-1.0
                    )
                    qqb = qq[:, :].unsqueeze(2).broadcast_to([P, TB, E])
                    p2b = p2[:, :].unsqueeze(2).broadcast_to([P, TB, E])
                    o1 = work.tile([P, FDB], f32)
                    o13 = o1[:, :].rearrange("c (t x) -> c t x", x=E)
                    ve.tensor_tensor(
                        out=o13, in0=st["c13"], in1=qqb, op=Alu.mult
                    )
                    ve.tensor_tensor(
                        out=res3, in0=st["mask23"], in1=p2b, op=Alu.mult
                    )
                    ve.tensor_tensor(
                        out=res[:, :], in0=o1[:, :], in1=res[:, :], op=Alu.add
                    )
                out_eng = getattr(nc, cfg["out_queue"])
                out_eng.dma_start(
                    out=ov[st["bi"]][:, lo:hi],
                    in_=res[:, :].rearrange("c (k q) -> c k q", k=hi - lo),
                )

            loop_ctx = (
                tc.For_i(0, repeats, 1)
                if repeats > 1
                else contextlib.nullcontext()
            )
            with loop_ctx:
              pending = None
              for bi in range(nbatch):
                sb_t0 = t0_pool.tile([P, FDB], f32)
                if pending is not None:
                    if cfg["final2_split"] and pending["bi"] == nbatch - 2:
                        # emit as two half-chains entirely up front so the
                        # end-of-stream DVE sequence is all half-width
                        for ck in (0, BATCH // 2):
                            s = dict(pending, lo=ck, hi=ck + BATCH // 2)
                            chain_prefix(s)
                            chain_suffix(s)
                        pending = None
                    else:
                        chain_prefix(pending)
                for gi in range(BATCH // GROUP):
                    psum_g = g_psum_pool.tile([P, GROUP * JT * E], f32)
                    for ci in range(GROUP):
                        c = (bi * (BATCH // GROUP) + gi) * GROUP + ci
                        h_sb = h_pool.tile([P, JT * D], f32)
                        if cfg["dma_halves"] == 2:
                            nc.sync.dma_start(
                                out=h_sb[:, 0:1024], in_=hv[c][:, 0:1024]
                            )
                            nc.sync.dma_start(
                                out=h_sb[:, 1024:2048], in_=hv[c][:, 1024:2048]
                            )
                        else:
                            nc.sync.dma_start(out=h_sb[:, :], in_=hv[c])
                        for q in range(2):
                            psum_hT = hT_psum_pool.tile([P, 1024], f32)
                            for m in range(8):
                                sl = slice(
                                    1024 * q + 128 * m,
                                    1024 * q + 128 * (m + 1),
                                )
                                nc.tensor.transpose(
                                    psum_hT[:, 128 * m : 128 * (m + 1)],
                                    h_sb[:, sl],
                                    ident_sb[:, :],
                                )
                            sb_hT = hT_sb_pool.tile([P, 1024], f32)
                            nc.scalar.activation(
                                sb_hT[:, :], psum_hT[:, :], Act.Copy
                            )
                            for m in range(8):
                                i = q * 8 + m
                                off = ci * JT * E + 12 * i
                                nc.tensor.matmul(
                                    psum_g[:, off : off + 12],
                                    sb_hT[:, 128 * m : 128 * (m + 1)],
                                    wg_sb[:, :],
                                    start=True,
                                    stop=True,
                                )
                            if warm:
                                nc.tensor.matmul(
                                    warm_ps[:, 0 : min(warm_n, P)],
                                    dummy_w[:, :],
                                    dummy_x[:, 0 : min(warm_n, P)],
                                    start=True,
                                    stop=True,
                                )
                    # bias add: psum group (2 chunks) -> compact sbuf
                    gt = GROUP * JT  # 64 tokens per partition per group
                    in3 = psum_g[:, :].rearrange("c (t x) -> c t x", x=E)
                    out3 = sb_t0[
                        :, gi * gt * E : (gi + 1) * gt * E
                    ].rearrange("c (t x) -> c t x", x=E)
                    bias3 = (
                        bias_sb[:, :]
                        .unsqueeze(1)
                        .broadcast_to([P, gt, E])
                    )
                    nc.vector.tensor_tensor(
                        out=out3, in0=in3, in1=bias3, op=Alu.add
                    )

                    if gi == 0 and pending is not None:
                        chain_suffix(pending)
                        pending = None

                ve = (
                    nc.gpsimd
                    if (cfg["pool_chains"] and bi % 2 == 1)
                    else nc.vector
                )
                pending = {
                    "bi": bi, "t0": sb_t0, "lo": 0, "hi": BATCH, "ve": ve,
                }
              if pending is not None:
                # Final batch: split into sub-chains so earlier chunks'
                # top-2 runs while later chunks' matmuls still stream.
                step = BATCH // cfg["final_split"]
                for ck in range(0, BATCH, step):
                    st = dict(pending, lo=ck, hi=ck + step)
                    chain_prefix(st)
                    chain_suffix(st)

            if warm:
                warm_sink = const_pool.tile([P, 1], f32)
                nc.scalar.activation(
                    warm_sink[:, :], warm_ps[:, 0:1], Act.Copy
                )
                warm_dram = nc.dram_tensor("warm_sink_d", [P, 1], f32)
                nc.sync.dma_start(out=warm_dram[:, :], in_=warm_sink[:, :])

    nc.finalize()
    return nc


def _aux_inputs(W, b):
    ident = np.eye(P, dtype=np.float32)
    # wglue[64*bb + e, 6*bb' + x] = W[x, e] iff bb == bb'
    wglue = np.zeros((P, 2 * E), dtype=np.float32)
    wglue[0:D, 0:E] = W.T.astype(np.float32)
    wglue[D : 2 * D, E : 2 * E] = W.T.astype(np.float32)
    biasr = np.tile(b.astype(np.float32)[None, :], (P, 1))
    return ident, wglue, biasr


_NC_CACHE = {}


def _get_nc(nsh, repeats=1, warm=False, warm_n=256, cfg=None):
    key = (nsh, repeats, warm, warm_n,
           tuple(sorted((cfg or {}).items())))
    if key not in _NC_CACHE:
        _NC_CACHE[key] = _build_nc(nsh, repeats, warm, warm_n, cfg)
    return _NC_CACHE[key]


def kernel(h, W, b):
    global LAST_RESULTS
    from concourse.bass_utils import run_bass_kernel_spmd

    h = np.ascontiguousarray(np.asarray(h, dtype=np.float32))
    W = np.asarray(W, dtype=np.float32)
    b = np.asarray(b, dtype=np.float32)
    n = h.shape[0]
    nsh = n // NCORES
    nc = _get_nc(nsh, warm=True, warm_n=256)
    ident, wglue, biasr = _aux_inputs(W, b)
    in_maps = []
    for i in range(NCORES):
        in_maps.append(
            {
                "h": h[i * nsh : (i + 1) * nsh],
                "ident": ident,
                "wglue": wglue,
                "biasr": biasr,
            }
        )
    trace = bool(int(os.environ.get("KERNEL_TRACE", "0")))
    res = run_bass_kernel_spmd(
        nc, in_maps, list(range(NCORES)), trace=trace
    )
    LAST_RESULTS = res
    outs = [res.results[i]["out"] for i in range(NCORES)]
    return np.concatenate(outs, axis=0)
